# revision 14
# baseline (speedup 1.0000x reference)
"""MultiHeadAttention (B=2, S=2048, D=1024, H=16, softmax over query axis)
on 8 TRN2 NeuronCores.

Sharding: core c handles batch b = c//4 and head-group hg = c%4 (4 heads,
d_local = 256). QKV weights row-sharded by head group, Wo column-sharded;
each core produces a partial [S, D] output, host sums the 4 partials per
batch and adds the output bias.

Design: ACT-bound two-pass pipeline. Every PE stage is tile-position paired so
per-matmul LDWEIGHTS streams hide under the partner tile's moving data
(row-group pairs for scores, col-group pairs for AV and projections —
HW-concurrent; the cost model charges them serially so TimelineSim
over-reports). Each head-pair runs two passes over kt: pass 0 computes
q-half-0 scores+exp (E0 kept in SBUF for all 16 kt) while V and the other
projections fill PE slack using their own PSUM tags in a pass-scoped pool;
pass 1 computes q-half-1 exps and runs the (one-kt-deferred) AV into the W
accumulators, whose pool only exists during pass 1. This keeps the score
slot rotation free of foreign write-after-read waits so ACT never starves,
and PSUM fits in 8 banks in every phase. exp runs at N=1024 with accum_out
giving the softmax normalizer for free. The q-projection bias is dropped
(softmax over q is invariant to per-k score offsets); only K keeps its
bias.

Optimization notes (2026-08-11 HW session; A/B repetition-slope protocol,
this kernel measured 212-257us/rep across sessions vs 189.4us documented —
absolute scale drifts between sessions, all comparisons below are
same-process A/B):
- HW microbenches: exp ACTIVATE N=1024 costs ~1115ns from PSUM src vs
  ~850ns from SBUF src (the ~266ns PSUM-read init does not pipeline);
  accum_out adds only ~31ns on this HW (cost model claims 187-279ns).
  ACT busy here ~153us and paces ~70% of the wall (halving exp width
  drops the wall ~40us). Row-group and col-group matmul pairs DO run
  concurrently (~207/241ns per pair-unit, N=512); a lone M=128 matmul is
  285ns (LDWEIGHTS exposed).
- DVE tensor_scalar with accum_out runs at 1x on HW (~1098ns for
  [128,1024] bf16 SBUF; the cost model wrongly grants it 4x). Moving z
  off ACT onto DVE regressed +27us. A Schraudolph fast-exp on DVE
  (f32 PSUM -> int16-bitcast-bf16, one tensor_scalar; rel_err 3.6e-3 at
  30/128 granules) also regressed +14-22us: DVE ops in the slot ring
  stall it. GPSIMD is unusable (no PSUM port, no free-dim reduce,
  tensor_scalar+accum fails walrus codegen).
- Structure is PSUM-bound everywhere: score ring + W accumulators fill
  all 8 banks in each phase, so N=2048 exps / merged ACTIVATEs /
  out-proj prefold are geometrically blocked (bf16 matmul PSUM output is
  TRN3-only; DVE-written PSUM can't be matmul-accumulated on TRN2).
- Neutral within noise (+-5us): deeper e1/small/out_sb rings. Upfront
  (non-interleaved) projections cost +32us. Pass-1 interleave=True
  (row-pair co-eligible fills) measured equal-or-faster than h0-first
  across two A/B runs (med +0.8/+8.6us in its favor) and was adopted.
- Adopted: pass-0-only Schraudolph on 16 granules (8 per head-pair,
  kt-even, one head per kt; SCH_* constants below). All-32-granule
  pass-0 offload is neutral (pass 0 flips DVE-bound at ~41us/m); g=24
  regresses (med -15us); any pass-1 offload regresses even with AV
  deferred by 2 kt (med -9us). g=16 balances pass-0 ACT (26.8us/m) vs
  DVE (~23us/m); rel_err 3.05e-3.
- Adopted: asymmetric Schraudolph split 7 (m0) / 9 (m1): m0's pass-0
  DVE also carries the 16 V-bias adds (~6.3us), so shifting two
  granules to m1's slack rebalances both phases under ACT pace.
  A/B med/min +8.2/+1.0 and +7.3/+2.9 us in its favor; rel_err 3.05e-3.
- Adopted: xT-chunk DMA issued before the per-chunk wq/wk DMAs (the
  last xT chunk gates the first exp; ~1MB of weights no longer delays
  it). A/B med/min/mean +5.2/+5.3/+1.5us. Deferring the woT DMA to the
  tail was NOT adopted (risks a tail stall; it already queues last).
  Split W-evac across ACT/DVE at m-transitions measured slightly
  negative and was rejected.
- Replacing the Schraudolph z tensor_scalar+accum with reduce_sum is
  dead-even (1x DVE ops are read-stream-bound; the scratch write is
  free) - not adopted. Moving m1's 9th granule from kt=7 (which shares
  its kt with a K-proj evac) to evac-free kt=11 REGRESSES med -9.5us:
  co-locating DVE bursts beats spreading them into the sched-free tail.
  Host-pre-broadcast bv (eliminating the gpsimd partition_broadcast and
  its Q7 library reload) is neutral - the broadcast hides under the
  ramp - not adopted. Swapping m1's late kt=14 granule to odd/evac
  kt=3 (both placement gradients agreed) measured mixed
  (med -3.1/min +5.7us) - not adopted; placements are at a measured
  optimum surrounded by coin flips.
- Headline, pooled over FOUR same-process A/B runs (20/20/30/18
  interleaved trials) vs the pre-session baseline: med-diff
  +6.9(partial)/+10.5/+5.1/-2.5us, min-diff +17.8/+3.8/+8.3/-1.2us
  per rep. Pooled estimate: ~3-5us faster per rep; three of four runs
  favor this kernel, the fourth is slightly negative within the
  +-3-10us session noise. Each adopted change individually won two
  consecutive A/B runs at adoption time. rel_err 3.048e-3 vs the
  2e-2 gate.
"""

import os

import numpy as np
import ml_dtypes

import concourse.tile as tile
from concourse import bacc, mybir
from concourse.bass_utils import run_bass_kernel_spmd

B, S, D, H = 2, 2048, 1024, 16
HD = D // H            # 64
NCORES = 8
HPC = H // (NCORES // B)   # heads per core = 4
DL = HPC * HD              # local head dims = 256
CT = D // 128              # 8 contraction tiles over D
ST = S // 16               # unused; kept for clarity
BF = mybir.dt.bfloat16
F32 = mybir.dt.float32
bf16 = ml_dtypes.bfloat16
Exp = mybir.ActivationFunctionType.Exp

_CACHE = {}
LAST_RESULT = None

# Pass-0-only Schraudolph fast-exp on DVE: exp(s/8) ~= bitcast_bf16(
# int16(C1*s + C2)). Restricted to qh=0 granules (pass 0), where the slot
# ring has 1.5-buffer lead and DVE has slack; pass-1 granules (tight
# bufs=2+W phase, vs-chain on DVE) measurably stall the pipeline. C2 is
# shifted -8 from the canonical 16256 to cancel the qh-asymmetric bias in
# z (numpy-validated: rel_err 3.6e-3 at 32 granules, tol 2e-2).
SCH_C1 = 184.6650390625 / 8.0
SCH_C2 = 16248.0
SCH_PAT = int(os.environ.get("MHA_G", "16"))


def _dve_groups():
    """Asymmetric 7/9 split: pass-0(m0) hosts all 16 V-bias adds on
    DVE, so it carries fewer Schraudolph granules than pass-0(m1),
    keeping both phases just under their ACT pace (A/B-confirmed
    med +8.2/+7.3us vs the symmetric 8/8 split)."""
    out = set()
    for m in range(2):
        for kt in range(16):
            if m == 0:
                take = kt % 2 == 0 and kt != 14   # 7 granules
            else:
                take = kt % 2 == 0 or kt == 7     # 9 granules
            if take:
                out.add((m, kt, (kt + m) % 2))
    return out


def _emit_body(nc, tc, aps):
    xT, wqT, wkT, wvT, woT, bk, bv, out = aps
    dve_set = _dve_groups()
    with tc.tile_pool(name="const", bufs=1) as cp:
        bk_sb = cp.tile([128, 2], F32)
        nc.sync.dma_start(bk_sb[:], bk)
        bv_row = cp.tile([1, DL], F32)
        nc.sync.dma_start(bv_row[:], bv)
        bvb = cp.tile([128, DL], F32)
        nc.gpsimd.partition_broadcast(bvb[:], bv_row[:])

        wq_c, wk_c, wv_c, xt_ch = [], [], [], []
        for c in range(CT):
            cslice = slice(c * 128, (c + 1) * 128)
            # x chunk DMA first: the last xT chunk gates the first exp
            # (ramp), and weights behind it per chunk cost nothing (each
            # 64KB wq/wk lands right after its 512KB xt chunk)
            tx = cp.tile([128, S], BF, tag=f"xt{c}", name=f"xt{c}")
            nc.sync.dma_start(tx[:], xT[cslice, :])
            tq = cp.tile([128, DL], BF, tag=f"wq{c}", name=f"wq{c}")
            tk = cp.tile([128, DL], BF, tag=f"wk{c}", name=f"wk{c}")
            nc.sync.dma_start(tq[:], wqT[cslice, :])
            nc.sync.dma_start(tk[:], wkT[cslice, :])
            xt_ch.append(tx)
            wq_c.append(tq)
            wk_c.append(tk)
        for c in range(CT):
            cslice = slice(c * 128, (c + 1) * 128)
            tv = cp.tile([128, DL], BF, tag=f"wv{c}", name=f"wv{c}")
            nc.sync.dma_start(tv[:], wvT[cslice, :])
            wv_c.append(tv)
        woT_sb = cp.tile([128, DL // 128, D], BF)
        nc.sync.dma_start(woT_sb[:], woT.rearrange("(c p) j -> p c j", p=128))

        qT_sb = [cp.tile([128, S], BF, tag=f"qT{m}", name=f"qT{m}")
                 for m in range(2)]
        kT_sb = [cp.tile([128, S], BF, tag=f"kT{m}", name=f"kT{m}")
                 for m in range(2)]
        v_sb = cp.tile([128, 16, DL], BF)
        wt_sb = [cp.tile([128, S], BF, tag=f"wt{m}", name=f"wt{m}")
                 for m in range(2)]

        with tc.tile_pool(name="e0sb", bufs=1) as e0p, \
             tc.tile_pool(name="e1sb", bufs=3) as e1p, \
             tc.tile_pool(name="small", bufs=4) as sp:

            z_sb = [cp.tile([128, 2, 16], F32, tag=f"z{qh}", name=f"z{qh}")
                    for qh in range(2)]

            def proj_half(pp, pqs, key, wc, dst, m, ch, half, bias_col):
                """Half of a 512-wide q/k projection chunk (4 of 8 c-tiles);
                col-group-paired matmuls (64|64 output halves) so LDWEIGHTS
                hides under the partner stream. Evacuates on half 1."""
                if half == 0:
                    pqs[key] = pp.tile([128, 512], F32, tag="pq", bufs=1,
                                       name="pq")
                pq = pqs[key]
                xs = slice(ch * 512, (ch + 1) * 512)
                for c in range(half * 4, half * 4 + 4):
                    st_ = (c == 0)
                    sp_ = (c == CT - 1)
                    nc.tensor.matmul(
                        pq[0:64, :],
                        wc[c][:, m * 128:m * 128 + 64],
                        xt_ch[c][:, xs], start=st_, stop=sp_)
                    nc.tensor.matmul(
                        pq[64:128, :],
                        wc[c][:, m * 128 + 64:(m + 1) * 128],
                        xt_ch[c][:, xs], start=st_, stop=sp_,
                        tile_position=(0, 64), skip_group_check=True)
                if half == 1:
                    dslc = dst[m][:, ch * 512:(ch + 1) * 512]
                    if bias_col is None:
                        nc.vector.tensor_copy(dslc, pq[:])
                    else:
                        nc.vector.tensor_scalar_add(dslc, pq[:], bias_col)
                    del pqs[key]

            def v_chunk(pp, st):
                """V projection for one 128-row s-tile, col-group-paired."""
                # padded to a full 2KB PSUM bank: matmul start=True marks
                # the whole 2KB-aligned zero-region pending, so sub-bank
                # neighbors would be clobbered
                pv = pp.tile([128, DL], F32, tag="pv", bufs=1, name="pv",
                             padded_shape=[128, 512])
                for c in range(CT):
                    st_ = (c == 0)
                    sp_ = (c == CT - 1)
                    nc.tensor.matmul(
                        pv[0:64, 0:DL],
                        xt_ch[c][:, st * 128:st * 128 + 64],
                        wv_c[c][:], start=st_, stop=sp_)
                    nc.tensor.matmul(
                        pv[64:128, 0:DL],
                        xt_ch[c][:, st * 128 + 64:(st + 1) * 128],
                        wv_c[c][:], start=st_, stop=sp_,
                        tile_position=(0, 64), skip_group_check=True)
                nc.vector.tensor_add(v_sb[:, st, :], pv[:, 0:DL], bvb[:])

            def slot_fill(pp, m, kt, qh, e_dst, z_col, bufs,
                          interleave):
                """Score matmuls for one (kt, q-half) into two single-head
                PSUM slots, then N=1024 exps with accum. With interleave the
                h0/h1 matmuls alternate (row-group pairs, HW-concurrent) —
                needs bufs=3 so both slots' buffer WARs release early enough
                for the pair to be co-eligible. With bufs=2 (pass 1, where W
                takes the other 4 banks) h0's matmuls go first so its exp
                starts on time; h1's fill hides under it."""
                ks = slice(kt * 128, (kt + 1) * 128)
                slots = [pp.tile([128, 1024], F32, tag="sp", bufs=bufs,
                                 name=f"slot{h}") for h in range(2)]

                def mm(h, ch):
                    qs = slice(qh * 1024 + ch * 512,
                               qh * 1024 + (ch + 1) * 512)
                    cs = slice(ch * 512, (ch + 1) * 512)
                    nc.tensor.matmul(
                        slots[h][:, cs], kT_sb[m][h * 64:(h + 1) * 64, ks],
                        qT_sb[m][h * 64:(h + 1) * 64, qs],
                        start=True, stop=True)

                order = ([(0, 0), (1, 0), (0, 1), (1, 1)] if interleave
                         else [(0, 0), (0, 1), (1, 0), (1, 1)])
                for h, ch in order:
                    mm(h, ch)
                for h in range(2):
                    if qh == 0 and (m, kt, h) in dve_set:
                        nc.vector.tensor_scalar(
                            e_dst[:, h, :].bitcast(mybir.dt.int16),
                            slots[h][:], SCH_C1, SCH_C2,
                            mybir.AluOpType.mult, mybir.AluOpType.add)
                        zscr = sp.tile([128, 1024], BF, tag="zscr",
                                       name="zscr")
                        nc.vector.tensor_scalar(
                            zscr[:], e_dst[:, h, :], 1.0, None,
                            mybir.AluOpType.mult, mybir.AluOpType.add,
                            accum_out=z_col(h))
                    else:
                        nc.scalar.activation(
                            e_dst[:, h, :], slots[h][:], Exp,
                            scale=float(1.0 / np.sqrt(HD)),
                            accum_out=z_col(h))

            def av(wps, vss, e0_kt, e1_kt, kt):
                for qh, e in ((0, e0_kt), (1, e1_kt)):
                    for ch in range(2):
                        qs = slice(ch * 512, (ch + 1) * 512)
                        nc.tensor.matmul(
                            wps[qh][0:64, ch * 512:(ch + 1) * 512],
                            vss[0][:], e[:, 0, qs],
                            start=(kt == 0), stop=(kt == 15))
                        nc.tensor.matmul(
                            wps[qh][64:128, ch * 512:(ch + 1) * 512],
                            vss[1][:], e[:, 1, qs],
                            start=(kt == 0), stop=(kt == 15),
                            tile_position=(0, 64), skip_group_check=True)

            # filler schedule for pass 0 of each pair: (kt, wc, dst, m, ch,
            # half, bias). Chunks are split into 4-c halves so no kt carries
            # more than ~2.2us of PE work; all deadlines (K chunk j before
            # kt=4j, m1 tiles before pair 1) hold.
            def half_sched(p):
                if p == 0:
                    # K m0 chunk j before kt=4j; Q m0 c2-3 before pass1;
                    # Q m1 c0-1 + K m1 c0 before pair 1's pass 0
                    seq = [("k", 0, 1), ("q", 0, 2), ("k", 0, 2),
                           ("q", 0, 3), ("k", 0, 3), ("q", 1, 0),
                           ("q", 1, 1), ("k", 1, 0)]
                else:
                    # K m1 chunk j before kt=4j; Q m1 c2-3 before pass 1
                    seq = [("k", 1, 1), ("q", 1, 2), ("k", 1, 2),
                           ("q", 1, 3), ("k", 1, 3)]
                out = {}
                for i, (w, m, ch) in enumerate(seq):
                    for half in range(2):
                        out[2 * i + half] = (w, m, ch, half)
                return out

            # ---- attention body: 2 pairs x (pass0 + pass1) x 16 kt ----
            pqs = {}
            for p in range(2):
                m = p
                e0_sb = e0p.tile([128, 16, 2, 1024], BF, tag="E0",
                                 name="E0")
                sched = half_sched(p)
                with tc.tile_pool(name="p0ps", bufs=1, space="PSUM") as pp:
                    if p == 0:
                        # lead-in: only what pass0/kt0 needs (Q m0 chunks
                        # 0-1, K m0 chunk 0)
                        for ch in range(2):
                            for half in range(2):
                                proj_half(pp, pqs, "lead", wq_c, qT_sb, 0,
                                          ch, half, None)
                        for half in range(2):
                            proj_half(pp, pqs, "lead", wk_c, kT_sb, 0, 0,
                                      half, bk_sb[:, 0:1])
                    for kt in range(16):
                        slot_fill(pp, m, kt, 0, e0_sb[:, kt],
                                  lambda h: z_sb[0][:, h, kt:kt + 1],
                                  bufs=3, interleave=True)
                        if p == 0:
                            v_chunk(pp, kt)
                        if kt in sched:
                            w, pm, ch, half = sched[kt]
                            wc = wq_c if w == "q" else wk_c
                            dst = qT_sb if w == "q" else kT_sb
                            bias = (None if w == "q"
                                    else bk_sb[:, pm:pm + 1])
                            proj_half(pp, pqs, (w, pm, ch), wc, dst, pm,
                                      ch, half, bias)
                with tc.tile_pool(name="p1ps", bufs=1, space="PSUM") as wp:
                    wps = [wp.tile([128, 1024], F32, tag=f"w{qh}", bufs=1,
                                   name=f"wps{qh}")
                           for qh in range(2)]
                    pend = None
                    for kt in range(16):
                        e1_kt = e1p.tile([128, 2, 1024], BF, tag="E1",
                                         name="E1")
                        slot_fill(wp, m, kt, 1, e1_kt,
                                  lambda h: z_sb[1][:, h, kt:kt + 1],
                                  bufs=2, interleave=True)
                        # AV for kt-1: exps and normalizers finished last
                        # iteration, so the in-order PE never stalls.
                        if pend is not None:
                            av(wps, *pend)
                        # normalizers for kt (consumed by next-iter AV)
                        vss = []
                        for h in range(2):
                            zt = sp.tile([128, 1], F32, tag=f"zt{h}",
                                         name="zt")
                            nc.vector.tensor_add(
                                zt[:], z_sb[0][:, h, kt:kt + 1],
                                z_sb[1][:, h, kt:kt + 1])
                            r = sp.tile([128, 1], F32, tag=f"r{h}",
                                        name="r")
                            nc.vector.reciprocal(r[:], zt[:])
                            vs = sp.tile([128, HD], BF, tag=f"vs{h}",
                                         name="vs")
                            nc.vector.tensor_scalar_mul(
                                vs[:], v_sb[:, kt, p * 128 + h * 64:
                                            p * 128 + (h + 1) * 64], r[:])
                            vss.append(vs)
                        pend = (vss, e0_sb[:, kt], e1_kt, kt)
                    av(wps, *pend)
                    # evacuate W for this pair
                    for qh in range(2):
                        nc.vector.tensor_copy(
                            wt_sb[p][:, qh * 1024:(qh + 1) * 1024],
                            wps[qh][:])

        # ---- output projection (partial over local heads) ----
        with tc.tile_pool(name="out_sb", bufs=4) as osb, \
             tc.tile_pool(name="ops", bufs=3, space="PSUM") as ops:
            for st in range(16):
                po = ops.tile([128, D], F32, tag="po")
                for ch in range(2):
                    for c in range(2):
                        js = slice(ch * 512, (ch + 1) * 512)
                        st_ = (c == 0)
                        sp_ = (c == 1)
                        nc.tensor.matmul(
                            po[0:64, js],
                            wt_sb[c][:, st * 128:st * 128 + 64],
                            woT_sb[:, c, js], start=st_, stop=sp_)
                        nc.tensor.matmul(
                            po[64:128, js],
                            wt_sb[c][:, st * 128 + 64:(st + 1) * 128],
                            woT_sb[:, c, js], start=st_, stop=sp_,
                            tile_position=(0, 64), skip_group_check=True)
                ob = osb.tile([128, D], BF, tag="ob")
                # split each evacuation across DVE and ACT so neither trails
                nc.vector.tensor_copy(ob[:, 0:512], po[:, 0:512])
                nc.scalar.copy(ob[:, 512:1024], po[:, 512:1024])
                nc.sync.dma_start(out[st * 128:(st + 1) * 128, :], ob[:])


def _build(reps=None, marker=False):
    """reps=None: single-shot kernel. reps=N: python-unrolled N repetitions
    (benchmarking only). marker adds a dummy input named by reps so
    differently-unrolled builds can't alias in any compile cache."""
    nc = bacc.Bacc("TRN2", target_bir_lowering=False, debug=False,
                   num_devices=NCORES)
    if marker:
        nc.dram_tensor(f"repmark{reps or 1}", [1, 1], F32,
                       kind="ExternalInput")
    xT = nc.dram_tensor("xT", [D, S], BF, kind="ExternalInput").ap()
    wqT = nc.dram_tensor("wqT", [D, DL], BF, kind="ExternalInput").ap()
    wkT = nc.dram_tensor("wkT", [D, DL], BF, kind="ExternalInput").ap()
    wvT = nc.dram_tensor("wvT", [D, DL], BF, kind="ExternalInput").ap()
    woT = nc.dram_tensor("woT", [DL, D], BF, kind="ExternalInput").ap()
    bk = nc.dram_tensor("bk", [128, 2], F32, kind="ExternalInput").ap()
    bv = nc.dram_tensor("bv", [1, DL], F32, kind="ExternalInput").ap()
    out = nc.dram_tensor("out", [S, D], BF, kind="ExternalOutput").ap()
    aps = (xT, wqT, wkT, wvT, woT, bk, bv, out)

    with tile.TileContext(nc) as tc:
        for _ in range(reps or 1):
            _emit_body(nc, tc, aps)

    nc.compile()
    return nc


def _get_nc():
    if "nc" not in _CACHE:
        _CACHE["nc"] = _build()
    return _CACHE["nc"]


def _make_in_maps(x, wq, bq, wk, bk, wv, bv, wo):
    xTs = [np.ascontiguousarray(x[b].T).astype(bf16) for b in range(B)]
    in_maps = []
    for core in range(NCORES):
        b, hg = core // (NCORES // B), core % (NCORES // B)
        rows = slice(hg * DL, (hg + 1) * DL)
        in_maps.append({
            "xT": xTs[b],
            "wqT": np.ascontiguousarray(wq[rows].T).astype(bf16),
            "wkT": np.ascontiguousarray(wk[rows].T).astype(bf16),
            "wvT": np.ascontiguousarray(wv[rows].T).astype(bf16),
            "woT": np.ascontiguousarray(wo[:, rows].T).astype(bf16),
            "bk": np.ascontiguousarray(bk[rows].reshape(2, 128).T),
            "bv": np.ascontiguousarray(bv[rows].reshape(1, DL)),
        })
    return in_maps


def kernel(x, wq, bq, wk, bk, wv, bv, wo, bo):
    global LAST_RESULT
    x = np.asarray(x, dtype=np.float32)
    wq, bq = np.asarray(wq, np.float32), np.asarray(bq, np.float32)
    wk, bk = np.asarray(wk, np.float32), np.asarray(bk, np.float32)
    wv, bv = np.asarray(wv, np.float32), np.asarray(bv, np.float32)
    wo, bo = np.asarray(wo, np.float32), np.asarray(bo, np.float32)

    nc = _get_nc()
    in_maps = _make_in_maps(x, wq, bq, wk, bk, wv, bv, wo)

    trace = os.environ.get("MHA_TRACE", "0") == "1"
    res = run_bass_kernel_spmd(nc, in_maps, core_ids=list(range(NCORES)),
                               trace=trace)
    LAST_RESULT = res

    out = np.zeros((B, S, D), np.float32)
    for core in range(NCORES):
        out[core // (NCORES // B)] += res.results[core]["out"].astype(np.float32)
    out += bo[None, None, :]
    return out

